# revision 45
# baseline (speedup 1.0000x reference)
"""DiGCNNet forward on 8 Trainium2 NeuronCores, data-parallel over batch.

Math (per batch b):
  adj = mean_t graph_sigs[b]                  # [30, 30]
  xw  = real[b] @ W                           # [30, 256]
  agg = adj^T @ xw + conv_bias                # [30, 256]
  h   = relu(agg)
  ns  = h @ pool_w + pool_b                   # [30]
  lg  = ns @ head_w^T + head_b                # [7]
  out = softmax(lg)

Per-core strategy (64 batches, 16 groups of 4):
  - graph_sigs quantized to uint8 (x255; the 1/(255 T) scale is folded
    into W) and host-transposed/padded to a fully partition-contiguous
    layout [(chunk, 4b x 32i), (g2, 32j, 64t)]: each 8-batch chunk is one
    dense [128 x 4KB] DMA and the T-reduce lands adj directly as
    [128(4b x 32i), 32(j)] bf16 -- no PE reduce, no scatter DMAs.
  - T-reduces alternate DVE tensor_reduce / GpSimd int16 add-tree so two
    engines share the serial reduce chain.
  - conv_bias enters via gs pad-row i=30 (value 4 -> "ones row" 256) and
    an rt pad-column x with x@W = 63.75*cb, so no bias matmul is needed.
  - xw: realT pre-padded/pre-interleaved bf16, 4 accumulating
    [128,128]x[128,256] matmuls; ACT evacuates PSUM -> bf16 SBUF.
  - agg: 4 concurrent diagonal tile_position matmuls (32x32 tiles).
  - relu+pool+reduce fused into ONE DVE scalar_tensor_tensor:
    ns = sum_d(max(agg,0) * pool_w) via accum_out.
  - head: [128,28] bf16 matmul into column g of a persistent logits PSUM
    tile; softmax tail once at the end (pool_b folded into head bias).
"""

from contextlib import ExitStack

import numpy as np
import ml_dtypes

import concourse.bacc as bacc
import concourse.bass as bass
import concourse.tile as tile
from concourse import mybir
from concourse.bass_utils import run_bass_kernel_spmd

F32 = mybir.dt.float32
BF16 = mybir.dt.bfloat16
U8 = mybir.dt.uint8
I16 = mybir.dt.int16

B, T, N = 512, 64, 30
F_IN, D, C = 512, 256, 7
NCORES = 8
BL = B // NCORES        # 64 batches per core
GPB = 4                 # batches per group
NG = BL // GPB          # 16 groups
NP = 32                 # padded nodes per batch
NJ = 32                 # padded j per batch
ROW = 2 * NJ * T        # 4096 gs elems per (chunk, partition)
GCH = 8                 # batches per gs DMA chunk
NCH = BL // GCH         # 8 gs chunks
GP_RED = (1, 3, 5, 7, 9, 11, 13, 15)  # groups reduced on gpsimd

# bf16 const blob column offsets (per partition)
_WT0 = 0                # wt: [128, 4*256]
_PWB = _WT0 + 4 * D     # pwb: [128, 256]
_HWB = _PWB + D         # hwblk: [128, 28]
_CBLOB = _HWB + GPB * C


def _build_nc():
    nc = bacc.Bacc(None, target_bir_lowering=False)

    NDIR = 2
    gsd = nc.dram_tensor("gsd", (NDIR * 128, ROW), U8, kind="ExternalInput")
    gsp = nc.dram_tensor("gsp", ((NCH - NDIR) * 128, ROW), U8, kind="ExternalInput")
    rt = nc.dram_tensor("rt", (128, 4 * BL * NP), BF16, kind="ExternalInput")
    cbl = nc.dram_tensor("cbl", (128, _CBLOB), BF16, kind="ExternalInput")
    f32a = nc.dram_tensor("f32a", (GPB * C, 1 + GPB), F32, kind="ExternalInput")
    f32b = nc.dram_tensor("f32b", (GPB, GPB * C), F32, kind="ExternalInput")
    out = nc.dram_tensor("out", (BL, C), F32, kind="ExternalOutput")

    with tile.TileContext(nc) as tc, ExitStack() as ctx:
        consts = ctx.enter_context(tc.tile_pool(name="consts", bufs=1))
        gsb_pool = ctx.enter_context(tc.tile_pool(name="gsb", bufs=4))
        tree_pool = ctx.enter_context(tc.tile_pool(name="tree", bufs=2))
        adjt_pool = ctx.enter_context(tc.tile_pool(name="adjt", bufs=4))
        xwb_pool = ctx.enter_context(tc.tile_pool(name="xwb", bufs=2))
        scr_pool = ctx.enter_context(tc.tile_pool(name="scr", bufs=2))
        ns_pool = ctx.enter_context(tc.tile_pool(name="ns", bufs=2))
        tail_pool = ctx.enter_context(tc.tile_pool(name="tail", bufs=1))
        xwp_pool = ctx.enter_context(
            tc.tile_pool(name="xwp", bufs=2, space=bass.MemorySpace.PSUM)
        )
        aggp_pool = ctx.enter_context(
            tc.tile_pool(name="aggp", bufs=2, space=bass.MemorySpace.PSUM)
        )
        logit_pool = ctx.enter_context(
            tc.tile_pool(name="logit", bufs=1, space=bass.MemorySpace.PSUM)
        )
        tailp_pool = ctx.enter_context(
            tc.tile_pool(name="tailp", bufs=2, space=bass.MemorySpace.PSUM)
        )

        # chunks 0-1 load directly (uint8, full t=64) on the sync ring so
        # the pipeline starts while the gpsimd accum ring spins up
        NDIR = 2
        gst = [None] * NCH

        def gs_direct(ch):
            t = gsb_pool.tile([128, 2, NJ, T], U8, tag="gsd")
            nc.sync.dma_start(
                t[:],
                gsd[ch * 128 : (ch + 1) * 128].rearrange(
                    "p (g j t) -> p g j t", g=2, j=NJ
                ),
            )
            gst[ch] = t

        gs_direct(0)
        cbl_sb = consts.tile([128, _CBLOB], BF16, tag="cbl")
        nc.sync.dma_start(cbl_sb[:], cbl[:])

        wt_sb = cbl_sb[:, _WT0 : _WT0 + 4 * D].rearrange("p (c d) -> p c d", c=4)
        pwb_sb = cbl_sb[:, _PWB : _PWB + D]
        hw_sb = cbl_sb[:, _HWB : _HWB + GPB * C]

        # realT: first quarter (batch cols for groups 0-3) lands early so
        # the xw->agg->pool chain starts ASAP; the rest follows chunk 1.
        rt_all = consts.tile([128, 4, BL * NP], BF16, tag="rt_all")
        rt_r = rt[:].rearrange("p (c m) -> p c m", c=4)
        Q = BL * NP // 4
        nc.sync.dma_start(rt_all[:, :, 0:Q], rt_r[:, :, 0:Q])
        gs_direct(1)
        nc.sync.dma_start(rt_all[:, :, Q:], rt_r[:, :, Q:])
        # tail-only f32 constants ride last
        f32a_sb = consts.tile([GPB * C, 1 + GPB], F32, tag="f32a")
        nc.sync.dma_start(f32a_sb[:], f32a[:])
        f32b_sb = consts.tile([GPB, GPB * C], F32, tag="f32b")
        nc.sync.dma_start(f32b_sb[:], f32b[:])
        hbb_sb = f32a_sb[:, 0:1]
        b7_sb = f32a_sb[:, 1 : 1 + GPB]
        b7t_sb = f32b_sb[:]

        # chunks 2-7: gpsimd cast (uint8->bf16) pair DMAs, second t-half
        # accumulating onto the first -- the DMA engines do the 64 -> 32
        # t-halving of the T-reduce for free
        HALF = ROW // 2
        for ch in range(NDIR, NCH):
            t = gsb_pool.tile([128, 2, NJ, T // 2], BF16, tag="gst")
            r0 = (ch - NDIR) * 128
            for th in range(2):
                nc.gpsimd.dma_start(
                    t[:],
                    gsp[r0 : r0 + 128, th * HALF : (th + 1) * HALF].rearrange(
                        "p (g j t) -> p g j t", g=2, j=NJ
                    ),
                    accum_op=(
                        mybir.AluOpType.bypass if th == 0 else mybir.AluOpType.add
                    ),
                )
            gst[ch] = t

        logits_t = logit_pool.tile([GPB * C, 512], F32, tag="logits")
        logits = logits_t[:, 0:NG]

        for g in range(NG):
            ch, h = divmod(g, 2)

            # ---- T-reduce on DVE -> [128, 32] bf16 (sum over t; chunk 0
            # reads raw uint8 64-wide, later chunks pre-paired 32-wide)
            adjt = adjt_pool.tile([128, NJ], BF16, tag="adjt")
            src_ap = gst[ch][:, h, :, :]
            with nc.allow_low_precision(reason="int sums fit bf16"):
                nc.vector.reduce_sum(
                    adjt[:].rearrange("p (j o) -> p j o", o=1),
                    src_ap,
                    axis=mybir.AxisListType.X,
                )

            # ---- xw: 4 accumulating [128,128] matmuls -> PSUM
            xwp_t = xwp_pool.tile([128, 512], F32, tag="xwp")
            xwp = xwp_t[:, 0:D]
            for c4 in range(4):
                nc.tensor.matmul(
                    xwp[:],
                    rt_all[:, c4, g * 128 : (g + 1) * 128],
                    wt_sb[:, c4, :],
                    start=(c4 == 0),
                    stop=(c4 == 3),
                )
            xwb = xwb_pool.tile([128, D], BF16, tag="xwb")
            nc.scalar.activation(xwb[:], xwp[:], mybir.ActivationFunctionType.Copy)

            # ---- agg: 4 concurrent diagonal tile_position matmuls
            # (conv_bias arrives via the gs/rt pad rows)
            aggp_t = aggp_pool.tile([128, 512], F32, tag="aggp")
            aggp = aggp_t[:, 0:D]
            for k in range(GPB):
                s = slice(NP * k, NP * (k + 1))
                nc.tensor.matmul(
                    aggp[s, :],
                    adjt[s, :],
                    xwb[s, :],
                    start=True,
                    stop=True,
                    tile_position=(NP * k, NP * k),
                    skip_group_check=True,
                )

            # ---- fused relu + pool + reduce on DVE (one op)
            scr = scr_pool.tile([128, D], BF16, tag="scr")
            ns = ns_pool.tile([128, 1], BF16, tag="ns")
            with nc.allow_low_precision(reason="fp32 accumulator, bf16 out"):
                nc.vector.scalar_tensor_tensor(
                    scr[:],
                    aggp[:],
                    0.0,
                    pwb_sb,
                    op0=mybir.AluOpType.max,
                    op1=mybir.AluOpType.mult,
                    accum_out=ns[:],
                )

            # ---- head: logits column g
            nc.tensor.matmul(
                logits[:, g : g + 1], hw_sb, ns[:], start=True, stop=True
            )

        # ---- softmax over the 7 classes (partition sub-blocks of 7)
        e_t = tail_pool.tile([GPB * C, NG], F32, tag="e")
        nc.scalar.activation(
            e_t[:], logits[:], mybir.ActivationFunctionType.Exp, bias=hbb_sb
        )
        sum_pt = tailp_pool.tile([GPB, 512], F32, tag="tailp")
        sum_p = sum_pt[:, 0:NG]
        nc.tensor.matmul(sum_p, b7_sb, e_t[:], start=True, stop=True)
        ssb_t = tail_pool.tile([GPB, NG], F32, tag="ssb")
        nc.vector.tensor_copy(ssb_t[:], sum_p)
        bcast_pt = tailp_pool.tile([GPB * C, 512], F32, tag="tailp")
        bcast_p = bcast_pt[:, 0:NG]
        nc.tensor.matmul(bcast_p, b7t_sb, ssb_t[:], start=True, stop=True)
        rs_t = tail_pool.tile([GPB * C, NG], F32, tag="rs")
        nc.vector.reciprocal(rs_t[:], bcast_p)
        res_t = tail_pool.tile([GPB * C, NG], F32, tag="res")
        nc.vector.tensor_mul(res_t[:], e_t[:], rs_t[:])
        nc.scalar.dma_start(out.rearrange("(g bi) c -> (bi c) g", bi=GPB), res_t[:])

    nc.compile()
    return nc


_NC_CACHE = None


def _get_nc():
    global _NC_CACHE
    if _NC_CACHE is None:
        _NC_CACHE = _build_nc()
    return _NC_CACHE


def _f32c(x):
    return np.ascontiguousarray(np.asarray(x, dtype=np.float32))


def _bf16(x):
    return np.ascontiguousarray(np.asarray(x).astype(ml_dtypes.bfloat16))


def _prepare_in_maps(real, graph_sigs, W, conv_bias, pool_w, pool_b, head_w, head_b):
    real = _f32c(real)
    graph_sigs = _f32c(graph_sigs)
    W = _f32c(W)
    pw = _f32c(pool_w)
    hw = _f32c(head_w)
    cb = _f32c(conv_bias)

    # gs is quantized x255 and the reduce skips the 1/T mean: fold both into W
    w_eff = W / np.float32(T * 255.0)
    wt = w_eff.reshape(4, 128, D).transpose(1, 0, 2).reshape(128, 4 * D)

    hwblk = np.zeros((128, GPB * C), dtype=np.float32)
    for k in range(GPB):
        hwblk[NP * k : NP * k + N, k * C : (k + 1) * C] = hw.T

    cblob = np.zeros((128, _CBLOB), dtype=np.float32)
    cblob[:, _WT0 : _WT0 + 4 * D] = wt
    cblob[:, _PWB : _PWB + D] = np.broadcast_to(pw, (128, D))
    cblob[:, _HWB : _HWB + GPB * C] = hwblk

    # conv_bias via pad row: gs pad-row value 4 -> adjt pad = 256, and
    # rt pad-column x with x @ W = (255*64/256) * cb so 256 * x@w_eff = cb
    if np.any(cb):
        x_cb, *_ = np.linalg.lstsq(W.T, 63.75 * cb, rcond=None)
    else:
        x_cb = np.zeros(F_IN, dtype=np.float32)

    # pool_b shifts every node score; fold into head bias
    hb_eff = _f32c(head_b) + np.float32(np.asarray(pool_b)) * hw.sum(axis=1)
    f32a = np.zeros((GPB * C, 1 + GPB), dtype=np.float32)
    f32a[:, 0] = np.tile(hb_eff, GPB)
    for k in range(GPB):
        f32a[k * C : (k + 1) * C, 1 + k] = 1.0
    f32b = np.ascontiguousarray(f32a[:, 1:].T)

    consts = {"cbl": _bf16(cblob), "f32a": f32a, "f32b": f32b}

    in_maps = []
    for c in range(NCORES):
        s = slice(c * BL, (c + 1) * BL)
        # gs: quantize, pad i->32 (row 30 = 4), pad j->32, regroup
        gq = np.rint(graph_sigs[s] * 255.0).astype(np.uint8)  # [BL, T, N, N]
        gpad = np.zeros((BL, NP, NJ, T), dtype=np.uint8)
        gpad[:, 0:N, 0:N] = gq.transpose(0, 2, 3, 1)
        gpad[:, N, 0:N, :] = 4
        # [ch, g2, k, i, j, th, t32] -> [ch, (k, i), th, g2, j, t32]
        g7 = gpad.reshape(NCH, 2, GPB, NP, NJ, 2, T // 2)
        g6 = g7.transpose(0, 2, 3, 5, 1, 4, 6)
        # rt: [128(f%128), (c4, b, np)] pre-interleaved; pad col 30 = x_cb
        rloc = real[s].transpose(2, 0, 1)  # [F_IN, BL, N]
        rpad = np.zeros((F_IN, BL, NP), dtype=np.float32)
        rpad[:, :, 0:N] = rloc
        rpad[:, :, N] = x_cb[:, None]
        rt2 = rpad.reshape(4, 128, BL * NP).transpose(1, 0, 2).reshape(128, -1)
        # chunks 0-1: plain [g, j, t64] rows; chunks 2-7: [th, g, j, t32]
        NDIR = 2
        g0 = gpad.reshape(NCH, 2, GPB, NP, NJ, T)[0:NDIR].transpose(0, 2, 3, 1, 4, 5)
        in_maps.append(
            {
                "gsd": np.ascontiguousarray(g0.reshape(NDIR * 128, ROW)),
                "gsp": np.ascontiguousarray(
                    g6[NDIR:].reshape((NCH - NDIR) * 128, ROW)
                ),
                "rt": _bf16(rt2),
                **consts,
            }
        )
    return in_maps


def kernel(real, imag, graph_sigs, W, conv_bias, pool_w, pool_b, head_w, head_b):
    del imag  # unused by the forward pass
    in_maps = _prepare_in_maps(
        real, graph_sigs, W, conv_bias, pool_w, pool_b, head_w, head_b
    )
    nc = _get_nc()
    res = run_bass_kernel_spmd(nc, in_maps, core_ids=list(range(NCORES)))
    return np.concatenate([res.results[c]["out"] for c in range(NCORES)], axis=0)


# revision 46
# speedup vs baseline: 1.0904x; 1.0904x over previous
"""DiGCNNet forward on 8 Trainium2 NeuronCores, data-parallel over batch.

Math (per batch b):
  adj = mean_t graph_sigs[b]                  # [30, 30]
  xw  = real[b] @ W                           # [30, 256]
  agg = adj^T @ xw + conv_bias                # [30, 256]
  h   = relu(agg)
  ns  = h @ pool_w + pool_b                   # [30]
  lg  = ns @ head_w^T + head_b                # [7]
  out = softmax(lg)

Per-core strategy (64 batches, 16 groups of 4):
  - graph_sigs quantized to uint8 (x255; the 1/(255 T) scale is folded
    into W) and host-transposed/padded to a fully partition-contiguous
    layout [(chunk, 4b x 32i), (g2, 32j, 64t)]: each 8-batch chunk is one
    dense [128 x 4KB] DMA and the T-reduce lands adj directly as
    [128(4b x 32i), 32(j)] bf16 -- no PE reduce, no scatter DMAs.
  - T-reduces alternate DVE tensor_reduce / GpSimd int16 add-tree so two
    engines share the serial reduce chain.
  - conv_bias enters via gs pad-row i=30 (value 4 -> "ones row" 256) and
    an rt pad-column x with x@W = 63.75*cb, so no bias matmul is needed.
  - xw: realT pre-padded/pre-interleaved bf16, 4 accumulating
    [128,128]x[128,256] matmuls; ACT evacuates PSUM -> bf16 SBUF.
  - agg: 4 concurrent diagonal tile_position matmuls (32x32 tiles).
  - relu+pool+reduce fused into ONE DVE scalar_tensor_tensor:
    ns = sum_d(max(agg,0) * pool_w) via accum_out.
  - head: [128,28] bf16 matmul into column g of a persistent logits PSUM
    tile; softmax tail once at the end (pool_b folded into head bias).
"""

from contextlib import ExitStack

import numpy as np
import ml_dtypes

import concourse.bacc as bacc
import concourse.bass as bass
import concourse.tile as tile
from concourse import mybir
from concourse.bass_utils import run_bass_kernel_spmd

F32 = mybir.dt.float32
BF16 = mybir.dt.bfloat16
U8 = mybir.dt.uint8
I16 = mybir.dt.int16

B, T, N = 512, 64, 30
F_IN, D, C = 512, 256, 7
NCORES = 8
BL = B // NCORES        # 64 batches per core
GPB = 4                 # batches per group
NG = BL // GPB          # 16 groups
NP = 32                 # padded nodes per batch
NJ = 32                 # padded j per batch
ROW = 2 * NJ * T        # 4096 gs elems per (chunk, partition)
GCH = 8                 # batches per gs DMA chunk
NCH = BL // GCH         # 8 gs chunks
GP_RED = (1, 3, 5, 7, 9, 11, 13, 15)  # groups reduced on gpsimd

# bf16 const blob column offsets (per partition)
_WT0 = 0                # wt: [128, 4*256]
_PWB = _WT0 + 4 * D     # pwb: [128, 256]
_HWB = _PWB + D         # hwblk: [128, 28]
_CBLOB = _HWB + GPB * C


def _build_nc():
    nc = bacc.Bacc(None, target_bir_lowering=False)

    NDIR = 3
    gsd = nc.dram_tensor("gsd", (NDIR * 128, ROW), U8, kind="ExternalInput")
    gsp = nc.dram_tensor("gsp", ((NCH - NDIR) * 128, ROW), U8, kind="ExternalInput")
    rt = nc.dram_tensor("rt", (128, 4 * BL * NP), BF16, kind="ExternalInput")
    cbl = nc.dram_tensor("cbl", (128, _CBLOB), BF16, kind="ExternalInput")
    f32a = nc.dram_tensor("f32a", (GPB * C, 1 + GPB), F32, kind="ExternalInput")
    f32b = nc.dram_tensor("f32b", (GPB, GPB * C), F32, kind="ExternalInput")
    out = nc.dram_tensor("out", (BL, C), F32, kind="ExternalOutput")

    with tile.TileContext(nc) as tc, ExitStack() as ctx:
        consts = ctx.enter_context(tc.tile_pool(name="consts", bufs=1))
        gsb_pool = ctx.enter_context(tc.tile_pool(name="gsb", bufs=4))
        tree_pool = ctx.enter_context(tc.tile_pool(name="tree", bufs=2))
        adjt_pool = ctx.enter_context(tc.tile_pool(name="adjt", bufs=4))
        xwb_pool = ctx.enter_context(tc.tile_pool(name="xwb", bufs=2))
        scr_pool = ctx.enter_context(tc.tile_pool(name="scr", bufs=2))
        ns_pool = ctx.enter_context(tc.tile_pool(name="ns", bufs=2))
        tail_pool = ctx.enter_context(tc.tile_pool(name="tail", bufs=1))
        xwp_pool = ctx.enter_context(
            tc.tile_pool(name="xwp", bufs=2, space=bass.MemorySpace.PSUM)
        )
        aggp_pool = ctx.enter_context(
            tc.tile_pool(name="aggp", bufs=2, space=bass.MemorySpace.PSUM)
        )
        logit_pool = ctx.enter_context(
            tc.tile_pool(name="logit", bufs=1, space=bass.MemorySpace.PSUM)
        )
        tailp_pool = ctx.enter_context(
            tc.tile_pool(name="tailp", bufs=2, space=bass.MemorySpace.PSUM)
        )

        # chunks 0-2 load directly (uint8, full t=64) on the sync ring so
        # the pipeline starts while the gpsimd accum ring spins up
        NDIR = 3
        gst = [None] * NCH

        def gs_direct(ch):
            t = gsb_pool.tile([128, 2, NJ, T], U8, tag="gsd")
            nc.sync.dma_start(
                t[:],
                gsd[ch * 128 : (ch + 1) * 128].rearrange(
                    "p (g j t) -> p g j t", g=2, j=NJ
                ),
            )
            gst[ch] = t

        gs_direct(0)
        cbl_sb = consts.tile([128, _CBLOB], BF16, tag="cbl")
        nc.sync.dma_start(cbl_sb[:], cbl[:])
        f32a_sb = consts.tile([GPB * C, 1 + GPB], F32, tag="f32a")
        nc.sync.dma_start(f32a_sb[:], f32a[:])
        f32b_sb = consts.tile([GPB, GPB * C], F32, tag="f32b")
        nc.sync.dma_start(f32b_sb[:], f32b[:])
        hbb_sb = f32a_sb[:, 0:1]
        b7_sb = f32a_sb[:, 1 : 1 + GPB]
        b7t_sb = f32b_sb[:]

        wt_sb = cbl_sb[:, _WT0 : _WT0 + 4 * D].rearrange("p (c d) -> p c d", c=4)
        pwb_sb = cbl_sb[:, _PWB : _PWB + D]
        hw_sb = cbl_sb[:, _HWB : _HWB + GPB * C]

        gs_direct(1)
        # realT resident in SBUF, one dense [128 x 16KB] DMA on sync queue
        rt_all = consts.tile([128, 4, BL * NP], BF16, tag="rt_all")
        nc.sync.dma_start(rt_all[:], rt[:].rearrange("p (c m) -> p c m", c=4))
        gs_direct(2)

        # chunks 3-7: gpsimd cast (uint8->bf16) pair DMAs, second t-half
        # accumulating onto the first -- the DMA engines do the 64 -> 32
        # t-halving of the T-reduce for free
        HALF = ROW // 2
        for ch in range(NDIR, NCH):
            t = gsb_pool.tile([128, 2, NJ, T // 2], BF16, tag="gst")
            r0 = (ch - NDIR) * 128
            for th in range(2):
                nc.gpsimd.dma_start(
                    t[:],
                    gsp[r0 : r0 + 128, th * HALF : (th + 1) * HALF].rearrange(
                        "p (g j t) -> p g j t", g=2, j=NJ
                    ),
                    accum_op=(
                        mybir.AluOpType.bypass if th == 0 else mybir.AluOpType.add
                    ),
                )
            gst[ch] = t

        logits_t = logit_pool.tile([GPB * C, 512], F32, tag="logits")
        logits = logits_t[:, 0:NG]

        for g in range(NG):
            ch, h = divmod(g, 2)

            # ---- T-reduce on DVE -> [128, 32] bf16 (sum over t; chunk 0
            # reads raw uint8 64-wide, later chunks pre-paired 32-wide)
            adjt = adjt_pool.tile([128, NJ], BF16, tag="adjt")
            src_ap = gst[ch][:, h, :, :]
            with nc.allow_low_precision(reason="int sums fit bf16"):
                nc.vector.reduce_sum(
                    adjt[:].rearrange("p (j o) -> p j o", o=1),
                    src_ap,
                    axis=mybir.AxisListType.X,
                )

            # ---- xw: 4 accumulating [128,128] matmuls -> PSUM
            xwp_t = xwp_pool.tile([128, 512], F32, tag="xwp")
            xwp = xwp_t[:, 0:D]
            for c4 in range(4):
                nc.tensor.matmul(
                    xwp[:],
                    rt_all[:, c4, g * 128 : (g + 1) * 128],
                    wt_sb[:, c4, :],
                    start=(c4 == 0),
                    stop=(c4 == 3),
                )
            xwb = xwb_pool.tile([128, D], BF16, tag="xwb")
            nc.scalar.activation(xwb[:], xwp[:], mybir.ActivationFunctionType.Copy)

            # ---- agg: 4 concurrent diagonal tile_position matmuls
            # (conv_bias arrives via the gs/rt pad rows)
            aggp_t = aggp_pool.tile([128, 512], F32, tag="aggp")
            aggp = aggp_t[:, 0:D]
            for k in range(GPB):
                s = slice(NP * k, NP * (k + 1))
                nc.tensor.matmul(
                    aggp[s, :],
                    adjt[s, :],
                    xwb[s, :],
                    start=True,
                    stop=True,
                    tile_position=(NP * k, NP * k),
                    skip_group_check=True,
                )

            # ---- fused relu + pool + reduce on DVE (one op)
            scr = scr_pool.tile([128, D], BF16, tag="scr")
            ns = ns_pool.tile([128, 1], BF16, tag="ns")
            with nc.allow_low_precision(reason="fp32 accumulator, bf16 out"):
                nc.vector.scalar_tensor_tensor(
                    scr[:],
                    aggp[:],
                    0.0,
                    pwb_sb,
                    op0=mybir.AluOpType.max,
                    op1=mybir.AluOpType.mult,
                    accum_out=ns[:],
                )

            # ---- head: logits column g
            nc.tensor.matmul(
                logits[:, g : g + 1], hw_sb, ns[:], start=True, stop=True
            )

        # ---- softmax over the 7 classes (partition sub-blocks of 7)
        e_t = tail_pool.tile([GPB * C, NG], F32, tag="e")
        nc.scalar.activation(
            e_t[:], logits[:], mybir.ActivationFunctionType.Exp, bias=hbb_sb
        )
        sum_pt = tailp_pool.tile([GPB, 512], F32, tag="tailp")
        sum_p = sum_pt[:, 0:NG]
        nc.tensor.matmul(sum_p, b7_sb, e_t[:], start=True, stop=True)
        ssb_t = tail_pool.tile([GPB, NG], F32, tag="ssb")
        nc.vector.tensor_copy(ssb_t[:], sum_p)
        bcast_pt = tailp_pool.tile([GPB * C, 512], F32, tag="tailp")
        bcast_p = bcast_pt[:, 0:NG]
        nc.tensor.matmul(bcast_p, b7t_sb, ssb_t[:], start=True, stop=True)
        rs_t = tail_pool.tile([GPB * C, NG], F32, tag="rs")
        nc.vector.reciprocal(rs_t[:], bcast_p)
        res_t = tail_pool.tile([GPB * C, NG], F32, tag="res")
        nc.vector.tensor_mul(res_t[:], e_t[:], rs_t[:])
        nc.scalar.dma_start(out.rearrange("(g bi) c -> (bi c) g", bi=GPB), res_t[:])

    nc.compile()
    return nc


_NC_CACHE = None


def _get_nc():
    global _NC_CACHE
    if _NC_CACHE is None:
        _NC_CACHE = _build_nc()
    return _NC_CACHE


def _f32c(x):
    return np.ascontiguousarray(np.asarray(x, dtype=np.float32))


def _bf16(x):
    return np.ascontiguousarray(np.asarray(x).astype(ml_dtypes.bfloat16))


def _prepare_in_maps(real, graph_sigs, W, conv_bias, pool_w, pool_b, head_w, head_b):
    real = _f32c(real)
    graph_sigs = _f32c(graph_sigs)
    W = _f32c(W)
    pw = _f32c(pool_w)
    hw = _f32c(head_w)
    cb = _f32c(conv_bias)

    # gs is quantized x255 and the reduce skips the 1/T mean: fold both into W
    w_eff = W / np.float32(T * 255.0)
    wt = w_eff.reshape(4, 128, D).transpose(1, 0, 2).reshape(128, 4 * D)

    hwblk = np.zeros((128, GPB * C), dtype=np.float32)
    for k in range(GPB):
        hwblk[NP * k : NP * k + N, k * C : (k + 1) * C] = hw.T

    cblob = np.zeros((128, _CBLOB), dtype=np.float32)
    cblob[:, _WT0 : _WT0 + 4 * D] = wt
    cblob[:, _PWB : _PWB + D] = np.broadcast_to(pw, (128, D))
    cblob[:, _HWB : _HWB + GPB * C] = hwblk

    # conv_bias via pad row: gs pad-row value 4 -> adjt pad = 256, and
    # rt pad-column x with x @ W = (255*64/256) * cb so 256 * x@w_eff = cb
    if np.any(cb):
        x_cb, *_ = np.linalg.lstsq(W.T, 63.75 * cb, rcond=None)
    else:
        x_cb = np.zeros(F_IN, dtype=np.float32)

    # pool_b shifts every node score; fold into head bias
    hb_eff = _f32c(head_b) + np.float32(np.asarray(pool_b)) * hw.sum(axis=1)
    f32a = np.zeros((GPB * C, 1 + GPB), dtype=np.float32)
    f32a[:, 0] = np.tile(hb_eff, GPB)
    for k in range(GPB):
        f32a[k * C : (k + 1) * C, 1 + k] = 1.0
    f32b = np.ascontiguousarray(f32a[:, 1:].T)

    consts = {"cbl": _bf16(cblob), "f32a": f32a, "f32b": f32b}

    in_maps = []
    for c in range(NCORES):
        s = slice(c * BL, (c + 1) * BL)
        # gs: quantize, pad i->32 (row 30 = 4), pad j->32, regroup
        gq = np.rint(graph_sigs[s] * 255.0).astype(np.uint8)  # [BL, T, N, N]
        gpad = np.zeros((BL, NP, NJ, T), dtype=np.uint8)
        gpad[:, 0:N, 0:N] = gq.transpose(0, 2, 3, 1)
        gpad[:, N, 0:N, :] = 4
        # [ch, g2, k, i, j, th, t32] -> [ch, (k, i), th, g2, j, t32]
        g7 = gpad.reshape(NCH, 2, GPB, NP, NJ, 2, T // 2)
        g6 = g7.transpose(0, 2, 3, 5, 1, 4, 6)
        # rt: [128(f%128), (c4, b, np)] pre-interleaved; pad col 30 = x_cb
        rloc = real[s].transpose(2, 0, 1)  # [F_IN, BL, N]
        rpad = np.zeros((F_IN, BL, NP), dtype=np.float32)
        rpad[:, :, 0:N] = rloc
        rpad[:, :, N] = x_cb[:, None]
        rt2 = rpad.reshape(4, 128, BL * NP).transpose(1, 0, 2).reshape(128, -1)
        # chunks 0-2: plain [g, j, t64] rows; chunks 3-7: [th, g, j, t32]
        NDIR = 3
        g0 = gpad.reshape(NCH, 2, GPB, NP, NJ, T)[0:NDIR].transpose(0, 2, 3, 1, 4, 5)
        in_maps.append(
            {
                "gsd": np.ascontiguousarray(g0.reshape(NDIR * 128, ROW)),
                "gsp": np.ascontiguousarray(
                    g6[NDIR:].reshape((NCH - NDIR) * 128, ROW)
                ),
                "rt": _bf16(rt2),
                **consts,
            }
        )
    return in_maps


def kernel(real, imag, graph_sigs, W, conv_bias, pool_w, pool_b, head_w, head_b):
    del imag  # unused by the forward pass
    in_maps = _prepare_in_maps(
        real, graph_sigs, W, conv_bias, pool_w, pool_b, head_w, head_b
    )
    nc = _get_nc()
    res = run_bass_kernel_spmd(nc, in_maps, core_ids=list(range(NCORES)))
    return np.concatenate([res.results[c]["out"] for c in range(NCORES)], axis=0)
